# revision 12
# baseline (speedup 1.0000x reference)
"""Trainium2 8-core kernel for biased-attention with sigmoid gating.

Reference computation (per batch b):
  q = heads(q_x @ Wq) * C**-0.5 ; k = heads(kv_x @ Wk) ; v = heads(kv_x @ Wv)
  a = softmax(q k^T + bias1 + bias2, axis=-1)
  o = (a @ v) gated by sigmoid(q_x @ Wg + bg), then @ Wo + bo

Shapes: B=2, Q=K=2048, CQ=CK=CV=256, H=8, C=32, CO=256.

Sharding: 8 cores = 2 batches x 4 query-quarters (512 rows each). Each core
computes all 8 heads for its rows; no cross-core communication is needed.
The dominant cost is streaming the two [B,H,Q,K] f32 bias tensors (67 MB per
core); bias2 is DMA'd onto bias1's SBUF tile with an inline CCE add so the
sum costs no engine time.
"""

import numpy as np

B, Q, K, CQ, H, C, CO = 2, 2048, 2048, 256, 8, 32, 256
HC = H * C  # 256
QS = Q // 4  # 512 query rows per core
N_CORES = 8
SCALE = float(C) ** -0.5

_CACHED = {}


def _build():
    import concourse.bass as bass
    import concourse.mybir as mybir
    import concourse.tile as tile
    from concourse import bacc
    from concourse.masks import make_identity

    f32 = mybir.dt.float32
    bf16 = mybir.dt.bfloat16
    AF = mybir.ActivationFunctionType
    ALU = mybir.AluOpType

    nc = bacc.Bacc(None, target_bir_lowering=False)

    qx = nc.declare_dram_parameter("qx", [QS, CQ], f32, isOutput=False)
    kvx = nc.declare_dram_parameter("kvx", [K, CQ], f32, isOutput=False)
    b1 = nc.declare_dram_parameter("b1", [H, QS, K], f32, isOutput=False)
    b2 = nc.declare_dram_parameter("b2", [H, QS, K], f32, isOutput=False)
    Wq = nc.declare_dram_parameter("Wq", [CQ, HC], f32, isOutput=False)
    Wk = nc.declare_dram_parameter("Wk", [CQ, HC], f32, isOutput=False)
    Wv = nc.declare_dram_parameter("Wv", [CQ, HC], f32, isOutput=False)
    Wg = nc.declare_dram_parameter("Wg", [CQ, HC], f32, isOutput=False)
    bg = nc.declare_dram_parameter("bg", [HC], f32, isOutput=False)
    Wo = nc.declare_dram_parameter("Wo", [HC, CO], f32, isOutput=False)
    bo = nc.declare_dram_parameter("bo", [CO], f32, isOutput=False)
    out = nc.declare_dram_parameter("out", [QS, CO], f32, isOutput=True)

    with tile.TileContext(nc) as tc:
        with (
            tc.tile_pool(name="singles", bufs=1) as singles,
            tc.tile_pool(name="stage", bufs=3) as stage,
            tc.tile_pool(name="bias", bufs=3) as biasp,
            tc.tile_pool(name="work", bufs=3) as work,
            tc.tile_pool(name="ework", bufs=3) as ework,
            tc.tile_pool(name="ps", bufs=1, space="PSUM") as psp,
        ):
            ident = singles.tile([128, 128], bf16)
            make_identity(nc, ident)

            # ---- weights: load f32, cast to bf16, split into 2 row-chunks ----
            wbf = {}
            for name, w in (("Wq", Wq), ("Wk", Wk), ("Wv", Wv), ("Wg", Wg), ("Wo", Wo)):
                wtile = singles.tile([128, 2, 256], bf16, tag=f"w_{name}")
                for ck in range(2):
                    wf = stage.tile([128, 256], f32, tag="wstage")
                    nc.sync.dma_start(out=wf, in_=w[ck * 128:(ck + 1) * 128, :])
                    nc.vector.tensor_copy(wtile[:, ck, :], wf)
                wbf[name] = wtile

            # broadcast bg / bo across partitions
            bg_bc = singles.tile([128, HC], f32, tag="bg")
            nc.sync.dma_start(out=bg_bc, in_=bg[:].partition_broadcast(128))
            bo_bc = singles.tile([128, CO], f32, tag="bo")
            nc.sync.dma_start(out=bo_bc, in_=bo[:].partition_broadcast(128))

            # ---- transpose inputs: qxT [256ck, 512q], kvxT [256ck, 2048k] (bf16) ----
            qxT = singles.tile([128, 2, QS], bf16, tag="qxT")
            for rt in range(QS // 128):  # 4 row tiles
                xf = stage.tile([128, CQ], f32, tag="xstage")
                nc.sync.dma_start(out=xf, in_=qx[rt * 128:(rt + 1) * 128, :])
                xb = stage.tile([128, CQ], bf16, tag="xbf")
                nc.vector.tensor_copy(xb, xf)
                for ck in range(2):
                    tp = psp.tile([128, 128], bf16, tag="et_ps", bufs=2)
                    nc.tensor.transpose(tp, xb[:, ck * 128:(ck + 1) * 128], ident)
                    nc.any.tensor_copy(qxT[:, ck, rt * 128:(rt + 1) * 128], tp)
            kvxT = singles.tile([128, 2, K], bf16, tag="kvxT")
            for rt in range(K // 128):  # 16 row tiles
                xf = stage.tile([128, CQ], f32, tag="xstage")
                nc.sync.dma_start(out=xf, in_=kvx[rt * 128:(rt + 1) * 128, :])
                xb = stage.tile([128, CQ], bf16, tag="xbf")
                nc.vector.tensor_copy(xb, xf)
                for ck in range(2):
                    tp = psp.tile([128, 128], bf16, tag="et_ps", bufs=2)
                    nc.tensor.transpose(tp, xb[:, ck * 128:(ck + 1) * 128], ident)
                    nc.any.tensor_copy(kvxT[:, ck, rt * 128:(rt + 1) * 128], tp)

            # ---- projections (bf16 matmuls, f32 psum) ----
            # Per-head transposed projections, heads stacked on the free dim
            # (PE operands must start at base partition 0/32/64, so a
            # 4-heads-per-128-partitions packing is not usable as lhsT).
            # QT [32c, 8h, 512q] scaled by C^-0.5 ; KT [32c, 8h, 2048k]
            QT = singles.tile([32, H, QS], bf16, tag="QT")
            for h in range(H):
                ps = psp.tile([128, QS, 1], f32, tag="scores", bufs=3)
                for ck in range(2):
                    nc.tensor.matmul(
                        ps[:32, :, 0],
                        wbf["Wq"][:, ck, h * 32:(h + 1) * 32],
                        qxT[:, ck, :],
                        start=(ck == 0),
                        stop=(ck == 1),
                    )
                nc.vector.tensor_scalar_mul(QT[:, h, :], ps[:32, :, 0], SCALE)
            KT = singles.tile([32, H, K], bf16, tag="KT")
            for h in range(H):
                for kc in range(4):
                    ps = psp.tile([128, 512, 1], f32, tag="scores", bufs=3)
                    for ck in range(2):
                        nc.tensor.matmul(
                            ps[:32, :, 0],
                            wbf["Wk"][:, ck, h * 32:(h + 1) * 32],
                            kvxT[:, ck, kc * 512:(kc + 1) * 512],
                            start=(ck == 0),
                            stop=(ck == 1),
                        )
                    nc.any.tensor_copy(KT[:, h, kc * 512:(kc + 1) * 512], ps[:32, :, 0])

            # V natural [128kr, 16kt, 256hc] bf16
            Vn = singles.tile([128, K // 128, HC], bf16, tag="Vn")
            for kt in range(K // 128):
                ps = psp.tile([128, HC, 1], f32, tag="scores", bufs=3)
                for ck in range(2):
                    nc.tensor.matmul(
                        ps[:, :, 0],
                        kvxT[:, ck, kt * 128:(kt + 1) * 128],
                        wbf["Wv"][:, ck, :],
                        start=(ck == 0),
                        stop=(ck == 1),
                    )
                nc.any.tensor_copy(Vn[:, kt, :], ps[:, :, 0])

            # G natural [128q, 4qt, 256hc] f32 = sigmoid(qx @ Wg + bg)
            Gn = singles.tile([128, 4, HC], f32, tag="Gn")
            for qt in range(4):
                ps = psp.tile([128, HC, 1], f32, tag="scores", bufs=3)
                for ck in range(2):
                    nc.tensor.matmul(
                        ps[:, :, 0],
                        qxT[:, ck, qt * 128:(qt + 1) * 128],
                        wbf["Wg"][:, ck, :],
                        start=(ck == 0),
                        stop=(ck == 1),
                    )
                gt = stage.tile([128, HC], f32, tag="gtmp")
                nc.vector.tensor_add(gt, ps[:, :, 0], bg_bc)
                nc.scalar.activation(Gn[:, qt, :], gt, AF.Sigmoid)

            # ---- main attention loops ----
            O_all = singles.tile([128, 4, HC], f32, tag="O_all")
            for h in range(H):
                hcc, hoff = h // 4, (h % 4) * 32
                hcol = h * 32
                for qt in range(4):
                    o_ps = psp.tile([128, C, 1], f32, tag="o_acc", bufs=2)
                    rs = work.tile([128, 4], f32, tag="rowsum")
                    Bsum = biasp.tile([128, K], f32, tag="bsum")
                    qsl = slice(qt * 128, (qt + 1) * 128)
                    nc.sync.dma_start(out=Bsum, in_=b1[h, qsl, :])
                    nc.gpsimd.dma_start(out=Bsum, in_=b2[h, qsl, :], accum_op=ALU.add)
                    for kc in range(4):
                        ksl = slice(kc * 512, (kc + 1) * 512)
                        s_ps = psp.tile([128, 512, 1], f32, tag="scores", bufs=3)
                        nc.tensor.matmul(
                            s_ps[:, :, 0],
                            QT[:, h, qsl],
                            KT[:, h, ksl],
                            start=True,
                            stop=True,
                        )
                        t_sb = work.tile([128, 512], f32, tag="t_add")
                        nc.vector.tensor_add(t_sb, s_ps[:, :, 0], Bsum[:, ksl])
                        e_sb = ework.tile([128, 512], bf16, tag="e")
                        nc.scalar.activation(
                            e_sb, t_sb, AF.Exp, accum_out=rs[:, kc:kc + 1]
                        )
                        et_ps = psp.tile([128, 4, 128], bf16, tag="et_ps", bufs=2)
                        for sub in range(4):
                            nc.tensor.transpose(
                                et_ps[:, sub, :], e_sb[:, sub * 128:(sub + 1) * 128], ident
                            )
                        et_sb = ework.tile([128, 4, 128], bf16, tag="et_sb")
                        nc.any.tensor_copy(et_sb, et_ps)
                        for sub in range(4):
                            nc.tensor.matmul(
                                o_ps[:, :, 0],
                                et_sb[:, sub, :],
                                Vn[:, kc * 4 + sub, hcol:hcol + 32],
                                start=(kc == 0 and sub == 0),
                                stop=(kc == 3 and sub == 3),
                            )
                    rsum = work.tile([128, 1], f32, tag="rsum")
                    junk = work.tile([128, 4], f32, tag="junk")
                    nc.scalar.activation(junk, rs, AF.Identity, accum_out=rsum)
                    rinv = work.tile([128, 1], f32, tag="rinv")
                    nc.vector.reciprocal(rinv, rsum)
                    nc.vector.tensor_scalar_mul(
                        O_all[:, qt, hcol:hcol + 32], o_ps[:, :, 0], rinv
                    )

            # ---- gating + output projection ----
            for qt in range(4):
                og = stage.tile([128, HC], bf16, tag="og")
                nc.vector.tensor_mul(og, O_all[:, qt, :], Gn[:, qt, :])
                ogt_ps = psp.tile([128, 2, 128], bf16, tag="et_ps", bufs=2)
                for hcc in range(2):
                    nc.tensor.transpose(
                        ogt_ps[:, hcc, :], og[:, hcc * 128:(hcc + 1) * 128], ident
                    )
                ogt = stage.tile([128, 2, 128], bf16, tag="ogt")
                nc.any.tensor_copy(ogt, ogt_ps)
                f_ps = psp.tile([128, CO, 1], f32, tag="scores", bufs=3)
                for hcc in range(2):
                    nc.tensor.matmul(
                        f_ps[:, :, 0],
                        ogt[:, hcc, :],
                        wbf["Wo"][:, hcc, :],
                        start=(hcc == 0),
                        stop=(hcc == 1),
                    )
                o_sb = stage.tile([128, CO], f32, tag="o_out")
                nc.vector.tensor_add(o_sb, f_ps[:, :, 0], bo_bc)
                nc.sync.dma_start(out=out[qt * 128:(qt + 1) * 128, :], in_=o_sb)

    nc.compile()
    return nc


def _get_nc():
    if "nc" not in _CACHED:
        _CACHED["nc"] = _build()
    return _CACHED["nc"]


def kernel(**inputs):
    from concourse.bass_utils import run_bass_kernel_spmd

    nc = _get_nc()
    inp = {k: np.asarray(v, dtype=np.float32) for k, v in inputs.items()}
    in_maps = []
    for c in range(N_CORES):
        b, qi = c // 4, c % 4
        q0 = qi * QS
        in_maps.append({
            "qx": np.ascontiguousarray(inp["q_x"][b, q0:q0 + QS, :]),
            "kvx": np.ascontiguousarray(inp["kv_x"][b]),
            "b1": np.ascontiguousarray(inp["bias1"][b, :, q0:q0 + QS, :]),
            "b2": np.ascontiguousarray(inp["bias2"][b, :, q0:q0 + QS, :]),
            "Wq": inp["Wq"], "Wk": inp["Wk"], "Wv": inp["Wv"], "Wg": inp["Wg"],
            "bg": inp["bg"], "Wo": inp["Wo"], "bo": inp["bo"],
        })
    res = run_bass_kernel_spmd(nc, in_maps, core_ids=list(range(N_CORES)))
    outa = np.empty((B, Q, CO), np.float32)
    for c in range(N_CORES):
        b, qi = c // 4, c % 4
        outa[b, qi * QS:(qi + 1) * QS, :] = res.results[c]["out"]
    return outa
